# revision 14
# baseline (speedup 1.0000x reference)
"""BasicRGCN (2-layer R-GCN conv + DistMult scoring) on 8 Trainium2 NeuronCores.

Sharding: entity rows N=4096 are split 512/core for the conv layers (the
[R,N,N] adjacency is the only large tensor; each core streams its row shard
as the matmul moving operand in transposed layout). Between layers the h
shards are AllGathered. The DistMult stage builds per-core shards of the
tables T[r] = H2 @ M_r (plus T[4] = H2), AllGathers them, and each core
resolves its 1024-sample batch shard with indirect-DMA row gathers.

Dataflow per core (SPMD, same program, different data):
  adjT chunks [32][128m, 4r, 512n] bf16 resident in SBUF (16 MiB)
  L1: aggT[r] = X_k.T @ adjT (PSUM accum over 32 k-chunks), deg via ones lhsT
      y.T = sum_r W1[r].T.T @ (aggT[r] * 1/deg[r]);  h1.T = sigmoid(y.T)
      PE-transpose -> h1 [512,128] bf16 -> AllGather -> H1 [4096,128]
  L2: same with H1/W2 -> h2.T [128f, 512n] f32
  T:  T[t][n,g] = h2T[:,nb].T @ relm5[t]  (relm5[4] = I) -> AllGather
  DistMult: row gathers from T_full by (rel,e1) and e2, fused mul+reduce.
"""

import sys
import types

import ml_dtypes
import numpy as np

N_CORES = 8
N_ENT = 4096
R = 4
F = 128
BATCH = 8192
NS = N_ENT // N_CORES  # 512 entity rows per core
BS = BATCH // N_CORES  # 1024 batch samples per core
KC = N_ENT // 128  # 32 contraction chunks
NB = NS // 128  # 4 row blocks per core
BF16 = ml_dtypes.bfloat16


def _install_ntff_shim():
    """Agent image's antenv lacks axon_hooks; recreate it from the boot's
    ctypes NTFF driver so trace=True profiling works when requested."""
    if "antenv.axon_hooks" in sys.modules:
        return
    try:
        import antenv
        from trn_agent_boot.trn_boot import _ntff_profile_via_ctypes

        hook = _ntff_profile_via_ctypes("/opt/axon/libaxon_pjrt.so")
        mod = types.ModuleType("antenv.axon_hooks")
        state = {"hook": hook}
        mod.get_axon_ntff_profile_hook = lambda: state["hook"]
        mod.set_axon_ntff_profile_hook = lambda h: state.__setitem__("hook", h)
        sys.modules["antenv.axon_hooks"] = mod
        antenv.axon_hooks = mod
    except Exception:
        pass


_NC = None


def _build(stage=4):
    from concourse import bacc, tile
    import concourse.mybir as mybir

    f32 = mybir.dt.float32
    bf16 = mybir.dt.bfloat16
    i32 = mybir.dt.int32
    Alu = mybir.AluOpType
    Act = mybir.ActivationFunctionType

    nc = bacc.Bacc(
        "TRN2", target_bir_lowering=False, debug=False, num_devices=N_CORES
    )

    adjt = nc.dram_tensor("adjt", [KC, 128, R, NS], bf16, kind="ExternalInput")
    feat = nc.dram_tensor("feat", [N_ENT, F], f32, kind="ExternalInput")
    w1t = nc.dram_tensor("w1t", [R, F, F], f32, kind="ExternalInput")
    w2t = nc.dram_tensor("w2t", [R, F, F], f32, kind="ExternalInput")
    relm5 = nc.dram_tensor("relm5", [R + 1, F, F], f32, kind="ExternalInput")
    e1x = nc.dram_tensor("e1x", [BS], i32, kind="ExternalInput")
    relx = nc.dram_tensor("relx", [BS], i32, kind="ExternalInput")
    e2x = nc.dram_tensor("e2x", [BS], i32, kind="ExternalInput")
    scores = nc.dram_tensor("scores", [BS], f32, kind="ExternalOutput")

    ag1_in_d = nc.dram_tensor("ag1_in", [NS, F], bf16)
    ag1_out_d = nc.dram_tensor("ag1_out", [N_CORES, NS, F], bf16)
    ag2_in_d = nc.dram_tensor("ag2_in", [R + 1, NS, F], f32)
    gtab = nc.dram_tensor("gtab", [N_CORES * (R + 1) * NS, F], f32)

    with tile.TileContext(nc) as tc:
        with (
            tc.tile_pool(name="adj", bufs=1) as adj_pool,
            tc.tile_pool(name="persist", bufs=1) as pp,
            tc.tile_pool(name="work", bufs=1) as wp,
            tc.tile_pool(name="psA", bufs=1, space="PSUM") as psA,
            tc.tile_pool(name="dram", bufs=1, space="DRAM") as dp,
        ):
            # ---------- constant / small loads ----------
            ones_bf = pp.tile([128, 128], bf16, tag="ones")
            nc.vector.memset(ones_bf[:], 1.0)

            w1t_sb = pp.tile([128, R, F], f32, tag="w1t")
            nc.sync.dma_start(out=w1t_sb[:], in_=w1t[:].rearrange("r f o -> f r o"))
            w2t_sb = pp.tile([128, R, F], f32, tag="w2t")
            nc.sync.dma_start(out=w2t_sb[:], in_=w2t[:].rearrange("r f o -> f r o"))
            relm_sb = pp.tile([128, R + 1, F], f32, tag="relm")
            nc.sync.dma_start(out=relm_sb[:], in_=relm5[:].rearrange("t f g -> f t g"))

            e1_sb = pp.tile([128, BS // 128], i32, tag="e1")
            nc.sync.dma_start(out=e1_sb[:], in_=e1x[:].rearrange("(p j) -> p j", p=128))
            rel_sb = pp.tile([128, BS // 128], i32, tag="rel")
            nc.sync.dma_start(
                out=rel_sb[:], in_=relx[:].rearrange("(p j) -> p j", p=128)
            )
            e2_sb = pp.tile([128, BS // 128], i32, tag="e2")
            nc.sync.dma_start(out=e2_sb[:], in_=e2x[:].rearrange("(p j) -> p j", p=128))

            # features: f32 load (staged), cast to bf16 chunk tiles
            x_bf = pp.tile([128, KC, F], bf16, tag="xbf")
            feat_v = feat[:].rearrange("(k p) f -> p k f", p=128)
            for q in range(4):
                x_f32 = wp.tile([128, KC // 4, F], f32, tag="xf32", bufs=2)
                nc.sync.dma_start(
                    out=x_f32[:], in_=feat_v[:, q * (KC // 4) : (q + 1) * (KC // 4), :]
                )
                nc.vector.tensor_copy(
                    out=x_bf[:, q * (KC // 4) : (q + 1) * (KC // 4), :], in_=x_f32[:]
                )

            # ---------- adjacency: 32 resident chunk tiles ----------
            adj_tiles = []
            for k in range(KC):
                t = adj_pool.tile([128, R, NS], bf16, tag=f"adj{k}")
                nc.sync.dma_start(out=t[:], in_=adjt[k])
                adj_tiles.append(t)

            # ---------- layer 1 matmuls: aggT[r] and deg[r] ----------
            agg_ps = [
                psA.tile([128, NS], f32, tag=f"agg{r}", name=f"agg{r}")
                for r in range(R)
            ]
            psD_cm = tc.tile_pool(name="psD", bufs=1, space="PSUM")
            psD = psD_cm.__enter__()
            deg_ps = [
                psD.tile([128, NS], f32, tag=f"deg{r}", name=f"deg{r}")
                for r in range(R)
            ]
            for k in range(KC):
                for r in range(R):
                    nc.tensor.matmul(
                        agg_ps[r][:],
                        x_bf[:, k, :],
                        adj_tiles[k][:, r, :],
                        start=(k == 0),
                        stop=(k == KC - 1),
                    )
                for r in range(R):
                    nc.tensor.matmul(
                        deg_ps[r][:],
                        ones_bf[:],
                        adj_tiles[k][:, r, :],
                        start=(k == 0),
                        stop=(k == KC - 1),
                    )

            # ---------- 1/deg with one Newton step (deg==0 -> 1) ----------
            invd = pp.tile([128, R, NS], f32, tag="invd")
            tmp = wp.tile([128, NS], f32, tag="nwt")
            for r in range(R):
                nc.vector.tensor_scalar_max(invd[:, r, :], deg_ps[r][:], 1.0)
                nc.vector.reciprocal(out=invd[:, r, :], in_=invd[:, r, :])
                # Newton: r1 = r0 * (2 - d*r0)
                nc.vector.tensor_tensor(
                    out=tmp[:], in0=deg_ps[r][:], in1=invd[:, r, :], op=Alu.mult
                )
                nc.vector.tensor_scalar(
                    out=tmp[:], in0=tmp[:], scalar1=-1.0, scalar2=2.0,
                    op0=Alu.mult, op1=Alu.add,
                )
                nc.vector.tensor_tensor(
                    out=invd[:, r, :], in0=invd[:, r, :], in1=tmp[:], op=Alu.mult
                )
            psD_cm.__exit__(None, None, None)
            psB_cm = tc.tile_pool(name="psB", bufs=1, space="PSUM")
            psB = psB_cm.__enter__()

            def conv_tail(agg_banks, wt_sb, out_ht):
                """scale by invd, m2 accumulate over r, sigmoid -> out_ht f32."""
                y_ps = psB.tile([128, NS], f32, tag="y")
                for r in range(R):
                    aggs = wp.tile([128, NS], f32, tag="aggs", bufs=2, name="aggs")
                    nc.vector.tensor_tensor(
                        out=aggs[:],
                        in0=agg_banks[r][:],
                        in1=invd[:, r, :],
                        op=Alu.mult,
                    )
                    nc.tensor.matmul(
                        y_ps[:],
                        wt_sb[:, r, :],
                        aggs[:],
                        start=(r == 0),
                        stop=(r == R - 1),
                    )
                nc.scalar.activation(out=out_ht[:], in_=y_ps[:], func=Act.Sigmoid)

            # ---------- layer 1 tail + transpose + AllGather ----------
            h1t = wp.tile([128, NS], f32, tag="h1t")
            conv_tail(agg_ps, w1t_sb, h1t)

            s_sb = wp.tile([128, BS // 128], f32, tag="ssb")
            if stage >= 2:
                ident = relm_sb[:, R, :]  # [128,128] identity (host-provided)
                h1n = wp.tile([128, NB, F], bf16, tag="h1n")
                for nb in range(NB):
                    tr_ps = psB.tile([128, F], f32, tag="tr")
                    nc.tensor.transpose(
                        tr_ps[:], h1t[:, nb * 128 : (nb + 1) * 128], ident
                    )
                    nc.vector.tensor_copy(out=h1n[:, nb, :], in_=tr_ps[:])

                nc.sync.dma_start(
                    out=ag1_in_d[:].rearrange("(nb p) f -> p nb f", p=128),
                    in_=h1n[:],
                )
                nc.gpsimd.collective_compute(
                    "AllGather",
                    Alu.bypass,
                    replica_groups=[list(range(N_CORES))],
                    ins=[ag1_in_d[:]],
                    outs=[ag1_out_d[:]],
                )
                h1_all = pp.tile([128, KC, F], bf16, tag="h1all")
                nc.sync.dma_start(
                    out=h1_all[:],
                    in_=ag1_out_d[:].rearrange("c (q p) f -> p (c q) f", p=128),
                )

                # ---------- layer 2 ----------
                agg2_ps = [
                    psA.tile([128, NS], f32, tag=f"agg{r}", name=f"agg2_{r}")
                    for r in range(R)
                ]
                for k in range(KC):
                    for r in range(R):
                        nc.tensor.matmul(
                            agg2_ps[r][:],
                            h1_all[:, k, :],
                            adj_tiles[k][:, r, :],
                            start=(k == 0),
                            stop=(k == KC - 1),
                        )
                h2t = wp.tile([128, NS], f32, tag="h2t")
                conv_tail(agg2_ps, w2t_sb, h2t)

            if stage >= 3:
                # ---------- T tables: T[t] = H2_shard @ relm5[t] ----------
                for t in range(R + 1):
                    tt_sb = wp.tile([128, NB, F], f32, tag="ttab", bufs=2)
                    for nb in range(NB):
                        t_ps = psB.tile([128, F], f32, tag="tps")
                        nc.tensor.matmul(
                            t_ps[:],
                            h2t[:, nb * 128 : (nb + 1) * 128],
                            relm_sb[:, t, :],
                            start=True,
                            stop=True,
                        )
                        nc.vector.tensor_copy(out=tt_sb[:, nb, :], in_=t_ps[:])
                    nc.sync.dma_start(
                        out=ag2_in_d[t].rearrange("(nb p) f -> p nb f", p=128),
                        in_=tt_sb[:],
                    )

                nc.gpsimd.collective_compute(
                    "AllGather",
                    Alu.bypass,
                    replica_groups=[list(range(N_CORES))],
                    ins=[ag2_in_d[:]],
                    outs=[gtab[:].rearrange("(c t n) f -> c t n f", c=N_CORES, t=R + 1)],
                )

            if stage >= 4:
                # ---------- DistMult batch shard ----------
                # row index in [(c t n), f] view: c*2560 + t*512 + n
                u_off = wp.tile([128, BS // 128], i32, tag="uoff")
                v_off = wp.tile([128, BS // 128], i32, tag="voff")
                t1 = wp.tile([128, BS // 128], i32, tag="it1")
                # u_off = (e1>>9)*2560 + (e1&511) + rel*512
                nc.vector.tensor_scalar(
                    out=u_off[:], in0=e1_sb[:], scalar1=9, scalar2=None,
                    op0=Alu.logical_shift_right,
                )
                nc.vector.tensor_scalar_mul(u_off[:], u_off[:], 2560)
                nc.vector.tensor_scalar(
                    out=t1[:], in0=e1_sb[:], scalar1=511, scalar2=None,
                    op0=Alu.bitwise_and,
                )
                nc.vector.tensor_tensor(
                    out=u_off[:], in0=u_off[:], in1=t1[:], op=Alu.add
                )
                nc.vector.tensor_scalar_mul(t1[:], rel_sb[:], 512)
                nc.vector.tensor_tensor(
                    out=u_off[:], in0=u_off[:], in1=t1[:], op=Alu.add
                )
                # v_off = (e2>>9)*2560 + (e2&511) + 4*512
                nc.vector.tensor_scalar(
                    out=v_off[:], in0=e2_sb[:], scalar1=9, scalar2=None,
                    op0=Alu.logical_shift_right,
                )
                nc.vector.tensor_scalar_mul(v_off[:], v_off[:], 2560)
                nc.vector.tensor_scalar(
                    out=t1[:], in0=e2_sb[:], scalar1=511, scalar2=None,
                    op0=Alu.bitwise_and,
                )
                nc.vector.tensor_tensor(
                    out=v_off[:], in0=v_off[:], in1=t1[:], op=Alu.add
                )
                nc.vector.tensor_scalar_add(v_off[:], v_off[:], 2048)

                table_rows = gtab[:]
                prod = wp.tile([128, F], f32, tag="prod")
                from concourse import bass as _bass

                for j in range(BS // 128):
                    u_t = wp.tile([128, F], f32, tag="ut", bufs=2)
                    v_t = wp.tile([128, F], f32, tag="vt", bufs=2)
                    nc.gpsimd.indirect_dma_start(
                        out=u_t[:],
                        out_offset=None,
                        in_=table_rows,
                        in_offset=_bass.IndirectOffsetOnAxis(
                            ap=u_off[:, j : j + 1], axis=0
                        ),
                    )
                    nc.gpsimd.indirect_dma_start(
                        out=v_t[:],
                        out_offset=None,
                        in_=table_rows,
                        in_offset=_bass.IndirectOffsetOnAxis(
                            ap=v_off[:, j : j + 1], axis=0
                        ),
                    )
                    nc.vector.tensor_tensor(
                        out=prod[:], in0=u_t[:], in1=v_t[:], op=Alu.mult
                    )
                    nc.vector.tensor_reduce(
                        out=s_sb[:, j : j + 1], in_=prod[:],
                        axis=mybir.AxisListType.X, op=Alu.add,
                    )
            elif stage == 3:
                g_t = wp.tile([128, BS // 128], f32, tag="g3")
                nc.sync.dma_start(out=g_t[:], in_=gtab[:128, : BS // 128])
                nc.vector.tensor_copy(out=s_sb[:], in_=g_t[:])
            elif stage == 2:
                nc.vector.tensor_copy(out=s_sb[:], in_=h2t[:, : BS // 128])
            else:
                nc.vector.tensor_copy(out=s_sb[:], in_=h1t[:, : BS // 128])

            nc.sync.dma_start(
                out=scores[:].rearrange("(p j) -> p j", p=128), in_=s_sb[:]
            )
            psB_cm.__exit__(None, None, None)

    nc.compile()
    return nc


_STAGE = 4


def _get_nc():
    global _NC
    if _NC is None:
        _install_ntff_shim()
        _NC = _build(_STAGE)
    return _NC


def _prep_in_maps(features, adj, W1, W2, relmats, e1_idx, rel_idx, e2_idx):
    features = np.asarray(features, dtype=np.float32)
    adj = np.asarray(adj, dtype=np.float32)
    W1 = np.asarray(W1, dtype=np.float32)
    W2 = np.asarray(W2, dtype=np.float32)
    relmats = np.asarray(relmats, dtype=np.float32)
    e1 = np.asarray(e1_idx).astype(np.int32)
    rel = np.asarray(rel_idx).astype(np.int32)
    e2 = np.asarray(e2_idx).astype(np.int32)

    # adjT[m, r, n] = adj[r, n, m]; 0/1 values -> exact in bf16
    adjT = np.ascontiguousarray(adj.transpose(2, 0, 1)).astype(BF16)
    w1t = np.ascontiguousarray(W1.transpose(0, 2, 1))
    w2t = np.ascontiguousarray(W2.transpose(0, 2, 1))
    relm5 = np.concatenate(
        [relmats, np.eye(F, dtype=np.float32)[None]], axis=0
    ).astype(np.float32)

    in_maps = []
    for c in range(N_CORES):
        sl = slice(c * NS, (c + 1) * NS)
        bsl = slice(c * BS, (c + 1) * BS)
        adj_c = np.ascontiguousarray(adjT[:, :, sl]).reshape(KC, 128, R, NS)
        in_maps.append(
            {
                "adjt": adj_c,
                "feat": features,
                "w1t": w1t,
                "w2t": w2t,
                "relm5": relm5,
                "e1x": np.ascontiguousarray(e1[bsl]),
                "relx": np.ascontiguousarray(rel[bsl]),
                "e2x": np.ascontiguousarray(e2[bsl]),
            }
        )
    return in_maps


def kernel(
    features, adj, W1, W2, relmats, e1_idx, rel_idx, e2_idx, _trace=False
):
    from concourse.bass_utils import run_bass_kernel_spmd

    nc = _get_nc()
    in_maps = _prep_in_maps(
        features, adj, W1, W2, relmats, e1_idx, rel_idx, e2_idx
    )
    try:
        res = run_bass_kernel_spmd(
            nc, in_maps, list(range(N_CORES)), trace=_trace
        )
    except Exception:
        # transient NRT device errors recover on retry
        res = run_bass_kernel_spmd(
            nc, in_maps, list(range(N_CORES)), trace=_trace
        )
    out = np.concatenate([res.results[c]["scores"] for c in range(N_CORES)])
    if _trace:
        kernel.last_results = res
    return out


# revision 15
# speedup vs baseline: 1.0300x; 1.0300x over previous
"""BasicRGCN (2-layer R-GCN conv + DistMult scoring) on 8 Trainium2 NeuronCores.

Sharding: entity rows N=4096 are split 512/core for the conv layers (the
[R,N,N] adjacency is the only large tensor; each core streams its row shard
as the matmul moving operand in transposed layout). Between layers the h
shards are AllGathered. The DistMult stage builds per-core shards of the
tables T[r] = H2 @ M_r (plus T[4] = H2), AllGathers them, and each core
resolves its 1024-sample batch shard with indirect-DMA row gathers.

Dataflow per core (SPMD, same program, different data):
  adjT chunks [32][128m, 4r, 512n] bf16 resident in SBUF (16 MiB)
  L1: aggT[r] = X_k.T @ adjT (PSUM accum over 32 k-chunks), deg via ones lhsT
      y.T = sum_r W1[r].T.T @ (aggT[r] * 1/deg[r]);  h1.T = sigmoid(y.T)
      PE-transpose -> h1 [512,128] bf16 -> AllGather -> H1 [4096,128]
  L2: same with H1/W2 -> h2.T [128f, 512n] f32
  T:  T[t][n,g] = h2T[:,nb].T @ relm5[t]  (relm5[4] = I) -> AllGather
  DistMult: row gathers from T_full by (rel,e1) and e2, fused mul+reduce.
"""

import sys
import types

import ml_dtypes
import numpy as np

N_CORES = 8
N_ENT = 4096
R = 4
F = 128
BATCH = 8192
NS = N_ENT // N_CORES  # 512 entity rows per core
BS = BATCH // N_CORES  # 1024 batch samples per core
KC = N_ENT // 128  # 32 contraction chunks
NB = NS // 128  # 4 row blocks per core
BF16 = ml_dtypes.bfloat16


def _install_ntff_shim():
    """Agent image's antenv lacks axon_hooks; recreate it from the boot's
    ctypes NTFF driver so trace=True profiling works when requested."""
    if "antenv.axon_hooks" in sys.modules:
        return
    try:
        import antenv
        from trn_agent_boot.trn_boot import _ntff_profile_via_ctypes

        hook = _ntff_profile_via_ctypes("/opt/axon/libaxon_pjrt.so")
        mod = types.ModuleType("antenv.axon_hooks")
        state = {"hook": hook}
        mod.get_axon_ntff_profile_hook = lambda: state["hook"]
        mod.set_axon_ntff_profile_hook = lambda h: state.__setitem__("hook", h)
        sys.modules["antenv.axon_hooks"] = mod
        antenv.axon_hooks = mod
    except Exception:
        pass


_NC = None


def _build(stage=4):
    from concourse import bacc, tile
    import concourse.mybir as mybir

    f32 = mybir.dt.float32
    bf16 = mybir.dt.bfloat16
    i32 = mybir.dt.int32
    Alu = mybir.AluOpType
    Act = mybir.ActivationFunctionType

    nc = bacc.Bacc(
        "TRN2", target_bir_lowering=False, debug=False, num_devices=N_CORES
    )

    adjt = nc.dram_tensor("adjt", [KC, 128, R, NS], bf16, kind="ExternalInput")
    feat = nc.dram_tensor("feat", [N_ENT, F], f32, kind="ExternalInput")
    w1t = nc.dram_tensor("w1t", [R, F, F], f32, kind="ExternalInput")
    w2t = nc.dram_tensor("w2t", [R, F, F], f32, kind="ExternalInput")
    relm5 = nc.dram_tensor("relm5", [R + 1, F, F], f32, kind="ExternalInput")
    e1x = nc.dram_tensor("e1x", [BS], i32, kind="ExternalInput")
    relx = nc.dram_tensor("relx", [BS], i32, kind="ExternalInput")
    e2x = nc.dram_tensor("e2x", [BS], i32, kind="ExternalInput")
    scores = nc.dram_tensor("scores", [BS], f32, kind="ExternalOutput")

    ag1_in_d = nc.dram_tensor("ag1_in", [NS, F], bf16)
    ag1_out_d = nc.dram_tensor("ag1_out", [N_CORES, NS, F], bf16)
    ag2_in_d = nc.dram_tensor("ag2_in", [R + 1, NS, F], bf16)
    gtab = nc.dram_tensor("gtab", [N_CORES * (R + 1) * NS, F], bf16)

    with tile.TileContext(nc) as tc:
        with (
            tc.tile_pool(name="adj", bufs=1) as adj_pool,
            tc.tile_pool(name="persist", bufs=1) as pp,
            tc.tile_pool(name="work", bufs=1) as wp,
            tc.tile_pool(name="psA", bufs=1, space="PSUM") as psA,
            tc.tile_pool(name="dram", bufs=1, space="DRAM") as dp,
        ):
            # ---------- constant / small loads ----------
            ones_bf = pp.tile([128, 128], bf16, tag="ones")
            nc.vector.memset(ones_bf[:], 1.0)

            # ---------- adjacency: 32 resident chunk tiles (queued first) ----------
            adj_tiles = []
            for k in range(KC):
                t = adj_pool.tile([128, R, NS], bf16, tag=f"adj{k}")
                nc.sync.dma_start(out=t[:], in_=adjt[k])
                adj_tiles.append(t)

            w1t_sb = pp.tile([128, R, F], f32, tag="w1t")
            nc.sync.dma_start(out=w1t_sb[:], in_=w1t[:].rearrange("r f o -> f r o"))
            w2t_sb = pp.tile([128, R, F], f32, tag="w2t")
            nc.sync.dma_start(out=w2t_sb[:], in_=w2t[:].rearrange("r f o -> f r o"))
            relm_sb = pp.tile([128, R + 1, F], f32, tag="relm")
            nc.sync.dma_start(out=relm_sb[:], in_=relm5[:].rearrange("t f g -> f t g"))

            e1_sb = pp.tile([128, BS // 128], i32, tag="e1")
            nc.sync.dma_start(out=e1_sb[:], in_=e1x[:].rearrange("(p j) -> p j", p=128))
            rel_sb = pp.tile([128, BS // 128], i32, tag="rel")
            nc.sync.dma_start(
                out=rel_sb[:], in_=relx[:].rearrange("(p j) -> p j", p=128)
            )
            e2_sb = pp.tile([128, BS // 128], i32, tag="e2")
            nc.sync.dma_start(out=e2_sb[:], in_=e2x[:].rearrange("(p j) -> p j", p=128))

            # features: f32 load (staged), cast to bf16 chunk tiles
            x_bf = pp.tile([128, KC, F], bf16, tag="xbf")
            feat_v = feat[:].rearrange("(k p) f -> p k f", p=128)
            for q in range(4):
                x_f32 = wp.tile([128, KC // 4, F], f32, tag="xf32", bufs=2)
                nc.sync.dma_start(
                    out=x_f32[:], in_=feat_v[:, q * (KC // 4) : (q + 1) * (KC // 4), :]
                )
                nc.vector.tensor_copy(
                    out=x_bf[:, q * (KC // 4) : (q + 1) * (KC // 4), :], in_=x_f32[:]
                )

            # ---------- layer 1 matmuls: aggT[r] and deg[r] ----------
            agg_ps = [
                psA.tile([128, NS], f32, tag=f"agg{r}", name=f"agg{r}")
                for r in range(R)
            ]
            psD_cm = tc.tile_pool(name="psD", bufs=1, space="PSUM")
            psD = psD_cm.__enter__()
            deg_ps = [
                psD.tile([128, NS], f32, tag=f"deg{r}", name=f"deg{r}")
                for r in range(R)
            ]
            for k in range(KC):
                for r in range(R):
                    nc.tensor.matmul(
                        deg_ps[r][:],
                        ones_bf[:],
                        adj_tiles[k][:, r, :],
                        start=(k == 0),
                        stop=(k == KC - 1),
                    )
                for r in range(R):
                    nc.tensor.matmul(
                        agg_ps[r][:],
                        x_bf[:, k, :],
                        adj_tiles[k][:, r, :],
                        start=(k == 0),
                        stop=(k == KC - 1),
                    )

            # ---------- 1/deg with one Newton step (deg==0 -> 1) ----------
            invd = pp.tile([128, R, NS], f32, tag="invd")
            for r in range(R):
                nc.vector.tensor_scalar_max(invd[:, r, :], deg_ps[r][:], 1.0)
                nc.vector.reciprocal(out=invd[:, r, :], in_=invd[:, r, :])
            psD_cm.__exit__(None, None, None)
            psB_cm = tc.tile_pool(name="psB", bufs=1, space="PSUM")
            psB = psB_cm.__enter__()

            def conv_tail(agg_banks, wt_sb, out_ht):
                """scale by invd, m2 accumulate over r, sigmoid -> out_ht f32."""
                y_ps = psB.tile([128, NS], f32, tag="y")
                for r in range(R):
                    aggs = wp.tile([128, NS], f32, tag="aggs", bufs=2, name="aggs")
                    nc.vector.tensor_tensor(
                        out=aggs[:],
                        in0=agg_banks[r][:],
                        in1=invd[:, r, :],
                        op=Alu.mult,
                    )
                    nc.tensor.matmul(
                        y_ps[:],
                        wt_sb[:, r, :],
                        aggs[:],
                        start=(r == 0),
                        stop=(r == R - 1),
                    )
                nc.scalar.activation(out=out_ht[:], in_=y_ps[:], func=Act.Sigmoid)

            # ---------- layer 1 tail + transpose + AllGather ----------
            h1t = wp.tile([128, NS], f32, tag="h1t")
            conv_tail(agg_ps, w1t_sb, h1t)

            s_sb = wp.tile([128, BS // 128], f32, tag="ssb")
            if stage >= 2:
                ident = relm_sb[:, R, :]  # [128,128] identity (host-provided)
                h1n = wp.tile([128, NB, F], bf16, tag="h1n")
                for nb in range(NB):
                    tr_ps = psB.tile([128, F], f32, tag="tr")
                    nc.tensor.transpose(
                        tr_ps[:], h1t[:, nb * 128 : (nb + 1) * 128], ident
                    )
                    nc.vector.tensor_copy(out=h1n[:, nb, :], in_=tr_ps[:])

                nc.sync.dma_start(
                    out=ag1_in_d[:].rearrange("(nb p) f -> p nb f", p=128),
                    in_=h1n[:],
                )
                nc.gpsimd.collective_compute(
                    "AllGather",
                    Alu.bypass,
                    replica_groups=[list(range(N_CORES))],
                    ins=[ag1_in_d[:]],
                    outs=[ag1_out_d[:]],
                )
                h1_all = pp.tile([128, KC, F], bf16, tag="h1all")
                nc.sync.dma_start(
                    out=h1_all[:],
                    in_=ag1_out_d[:].rearrange("c (q p) f -> p (c q) f", p=128),
                )

                # ---------- layer 2 ----------
                agg2_ps = [
                    psA.tile([128, NS], f32, tag=f"agg{r}", name=f"agg2_{r}")
                    for r in range(R)
                ]
                for k in range(KC):
                    for r in range(R):
                        nc.tensor.matmul(
                            agg2_ps[r][:],
                            h1_all[:, k, :],
                            adj_tiles[k][:, r, :],
                            start=(k == 0),
                            stop=(k == KC - 1),
                        )
                h2t = wp.tile([128, NS], f32, tag="h2t")
                conv_tail(agg2_ps, w2t_sb, h2t)

            if stage >= 3:
                # ---------- T tables: T[t] = H2_shard @ relm5[t] ----------
                for t in range(R + 1):
                    tt_sb = wp.tile([128, NB, F], bf16, tag="ttab", bufs=2)
                    for nb in range(NB):
                        t_ps = psB.tile([128, F], f32, tag="tps")
                        nc.tensor.matmul(
                            t_ps[:],
                            h2t[:, nb * 128 : (nb + 1) * 128],
                            relm_sb[:, t, :],
                            start=True,
                            stop=True,
                        )
                        nc.vector.tensor_copy(out=tt_sb[:, nb, :], in_=t_ps[:])
                    nc.sync.dma_start(
                        out=ag2_in_d[t].rearrange("(nb p) f -> p nb f", p=128),
                        in_=tt_sb[:],
                    )

                nc.gpsimd.collective_compute(
                    "AllGather",
                    Alu.bypass,
                    replica_groups=[list(range(N_CORES))],
                    ins=[ag2_in_d[:]],
                    outs=[gtab[:].rearrange("(c t n) f -> c t n f", c=N_CORES, t=R + 1)],
                )

            if stage >= 4:
                # ---------- DistMult batch shard ----------
                # row index in [(c t n), f] view: c*2560 + t*512 + n
                u_off = wp.tile([128, BS // 128], i32, tag="uoff")
                v_off = wp.tile([128, BS // 128], i32, tag="voff")
                t1 = wp.tile([128, BS // 128], i32, tag="it1")
                # u_off = (e1>>9)*2560 + (e1&511) + rel*512
                nc.vector.tensor_scalar(
                    out=u_off[:], in0=e1_sb[:], scalar1=9, scalar2=None,
                    op0=Alu.logical_shift_right,
                )
                nc.vector.tensor_scalar_mul(u_off[:], u_off[:], 2560)
                nc.vector.tensor_scalar(
                    out=t1[:], in0=e1_sb[:], scalar1=511, scalar2=None,
                    op0=Alu.bitwise_and,
                )
                nc.vector.tensor_tensor(
                    out=u_off[:], in0=u_off[:], in1=t1[:], op=Alu.add
                )
                nc.vector.tensor_scalar_mul(t1[:], rel_sb[:], 512)
                nc.vector.tensor_tensor(
                    out=u_off[:], in0=u_off[:], in1=t1[:], op=Alu.add
                )
                # v_off = (e2>>9)*2560 + (e2&511) + 4*512
                nc.vector.tensor_scalar(
                    out=v_off[:], in0=e2_sb[:], scalar1=9, scalar2=None,
                    op0=Alu.logical_shift_right,
                )
                nc.vector.tensor_scalar_mul(v_off[:], v_off[:], 2560)
                nc.vector.tensor_scalar(
                    out=t1[:], in0=e2_sb[:], scalar1=511, scalar2=None,
                    op0=Alu.bitwise_and,
                )
                nc.vector.tensor_tensor(
                    out=v_off[:], in0=v_off[:], in1=t1[:], op=Alu.add
                )
                nc.vector.tensor_scalar_add(v_off[:], v_off[:], 2048)

                table_rows = gtab[:]
                prod = wp.tile([128, F], f32, tag="prod")
                from concourse import bass as _bass

                for j in range(BS // 128):
                    u_t = wp.tile([128, F], bf16, tag="ut", bufs=2)
                    v_t = wp.tile([128, F], bf16, tag="vt", bufs=2)
                    nc.gpsimd.indirect_dma_start(
                        out=u_t[:],
                        out_offset=None,
                        in_=table_rows,
                        in_offset=_bass.IndirectOffsetOnAxis(
                            ap=u_off[:, j : j + 1], axis=0
                        ),
                    )
                    nc.gpsimd.indirect_dma_start(
                        out=v_t[:],
                        out_offset=None,
                        in_=table_rows,
                        in_offset=_bass.IndirectOffsetOnAxis(
                            ap=v_off[:, j : j + 1], axis=0
                        ),
                    )
                    nc.vector.tensor_tensor(
                        out=prod[:], in0=u_t[:], in1=v_t[:], op=Alu.mult
                    )
                    nc.vector.tensor_reduce(
                        out=s_sb[:, j : j + 1], in_=prod[:],
                        axis=mybir.AxisListType.X, op=Alu.add,
                    )
            elif stage == 3:
                g_t = wp.tile([128, BS // 128], bf16, tag="g3")
                nc.sync.dma_start(out=g_t[:], in_=gtab[:128, : BS // 128])
                nc.vector.tensor_copy(out=s_sb[:], in_=g_t[:])
            elif stage == 2:
                nc.vector.tensor_copy(out=s_sb[:], in_=h2t[:, : BS // 128])
            else:
                nc.vector.tensor_copy(out=s_sb[:], in_=h1t[:, : BS // 128])

            nc.sync.dma_start(
                out=scores[:].rearrange("(p j) -> p j", p=128), in_=s_sb[:]
            )
            psB_cm.__exit__(None, None, None)

    nc.compile()
    return nc


_STAGE = 4


def _get_nc():
    global _NC
    if _NC is None:
        _install_ntff_shim()
        _NC = _build(_STAGE)
    return _NC


def _prep_in_maps(features, adj, W1, W2, relmats, e1_idx, rel_idx, e2_idx):
    features = np.asarray(features, dtype=np.float32)
    adj = np.asarray(adj, dtype=np.float32)
    W1 = np.asarray(W1, dtype=np.float32)
    W2 = np.asarray(W2, dtype=np.float32)
    relmats = np.asarray(relmats, dtype=np.float32)
    e1 = np.asarray(e1_idx).astype(np.int32)
    rel = np.asarray(rel_idx).astype(np.int32)
    e2 = np.asarray(e2_idx).astype(np.int32)

    # adjT[m, r, n] = adj[r, n, m]; 0/1 values -> exact in bf16
    adjT = np.ascontiguousarray(adj.transpose(2, 0, 1)).astype(BF16)
    w1t = np.ascontiguousarray(W1.transpose(0, 2, 1))
    w2t = np.ascontiguousarray(W2.transpose(0, 2, 1))
    relm5 = np.concatenate(
        [relmats, np.eye(F, dtype=np.float32)[None]], axis=0
    ).astype(np.float32)

    in_maps = []
    for c in range(N_CORES):
        sl = slice(c * NS, (c + 1) * NS)
        bsl = slice(c * BS, (c + 1) * BS)
        adj_c = np.ascontiguousarray(adjT[:, :, sl]).reshape(KC, 128, R, NS)
        in_maps.append(
            {
                "adjt": adj_c,
                "feat": features,
                "w1t": w1t,
                "w2t": w2t,
                "relm5": relm5,
                "e1x": np.ascontiguousarray(e1[bsl]),
                "relx": np.ascontiguousarray(rel[bsl]),
                "e2x": np.ascontiguousarray(e2[bsl]),
            }
        )
    return in_maps


def kernel(
    features, adj, W1, W2, relmats, e1_idx, rel_idx, e2_idx, _trace=False
):
    from concourse.bass_utils import run_bass_kernel_spmd

    nc = _get_nc()
    in_maps = _prep_in_maps(
        features, adj, W1, W2, relmats, e1_idx, rel_idx, e2_idx
    )
    try:
        res = run_bass_kernel_spmd(
            nc, in_maps, list(range(N_CORES)), trace=_trace
        )
    except Exception:
        # transient NRT device errors recover on retry
        res = run_bass_kernel_spmd(
            nc, in_maps, list(range(N_CORES)), trace=_trace
        )
    out = np.concatenate([res.results[c]["scores"] for c in range(N_CORES)])
    if _trace:
        kernel.last_results = res
    return out


# revision 19
# speedup vs baseline: 1.1029x; 1.0707x over previous
"""BasicRGCN (2-layer R-GCN conv + DistMult scoring) on 8 Trainium2 NeuronCores.

Sharding: entity rows N=4096 are split 512/core for the conv layers (the
[R,N,N] adjacency is the only large tensor; each core streams its row shard
as the matmul moving operand in transposed layout). Between layers the h
shards are AllGathered. The DistMult stage builds per-core shards of the
tables T[r] = H2 @ M_r (plus T[4] = H2), AllGathers them, and each core
resolves its 1024-sample batch shard with indirect-DMA row gathers.

Dataflow per core (SPMD, same program, different data):
  adjT chunks [32][128m, 4r, 512n] bf16 resident in SBUF (16 MiB)
  L1: aggT[r] = X_k.T @ adjT (PSUM accum over 32 k-chunks), deg via ones lhsT
      y.T = sum_r W1[r].T.T @ (aggT[r] * 1/deg[r]);  h1.T = sigmoid(y.T)
      PE-transpose -> h1 [512,128] bf16 -> AllGather -> H1 [4096,128]
  L2: same with H1/W2 -> h2.T [128f, 512n] f32
  T:  T[t][n,g] = h2T[:,nb].T @ relm5[t]  (relm5[4] = I) -> AllGather
  DistMult: row gathers from T_full by (rel,e1) and e2, fused mul+reduce.
"""

import sys
import types

import ml_dtypes
import numpy as np

N_CORES = 8
N_ENT = 4096
R = 4
F = 128
BATCH = 8192
NS = N_ENT // N_CORES  # 512 entity rows per core
BS = BATCH // N_CORES  # 1024 batch samples per core
KC = N_ENT // 128  # 32 contraction chunks
NB = NS // 128  # 4 row blocks per core
BF16 = ml_dtypes.bfloat16


def _install_ntff_shim():
    """Agent image's antenv lacks axon_hooks; recreate it from the boot's
    ctypes NTFF driver so trace=True profiling works when requested."""
    if "antenv.axon_hooks" in sys.modules:
        return
    try:
        import antenv
        from trn_agent_boot.trn_boot import _ntff_profile_via_ctypes

        hook = _ntff_profile_via_ctypes("/opt/axon/libaxon_pjrt.so")
        mod = types.ModuleType("antenv.axon_hooks")
        state = {"hook": hook}
        mod.get_axon_ntff_profile_hook = lambda: state["hook"]
        mod.set_axon_ntff_profile_hook = lambda h: state.__setitem__("hook", h)
        sys.modules["antenv.axon_hooks"] = mod
        antenv.axon_hooks = mod
    except Exception:
        pass


_NC = None


def _build(stage=4):
    from concourse import bacc, tile
    import concourse.mybir as mybir

    f32 = mybir.dt.float32
    bf16 = mybir.dt.bfloat16
    i32 = mybir.dt.int32
    Alu = mybir.AluOpType
    Act = mybir.ActivationFunctionType

    nc = bacc.Bacc(
        "TRN2", target_bir_lowering=False, debug=False, num_devices=N_CORES
    )

    adjt = nc.dram_tensor("adjt", [KC, 128, R, NS], bf16, kind="ExternalInput")
    feat = nc.dram_tensor("feat", [N_ENT, F], f32, kind="ExternalInput")
    w1t = nc.dram_tensor("w1t", [R, F, F], f32, kind="ExternalInput")
    w2t = nc.dram_tensor("w2t", [R, F, F], f32, kind="ExternalInput")
    relm5 = nc.dram_tensor("relm5", [R + 1, F, F], f32, kind="ExternalInput")
    e1x = nc.dram_tensor("e1x", [BS], i32, kind="ExternalInput")
    relx = nc.dram_tensor("relx", [BS], i32, kind="ExternalInput")
    e2x = nc.dram_tensor("e2x", [BS], i32, kind="ExternalInput")
    scores = nc.dram_tensor("scores", [BS], f32, kind="ExternalOutput")

    ag1_in_d = nc.dram_tensor("ag1_in", [NS, F], bf16)
    ag1_out_d = nc.dram_tensor("ag1_out", [N_CORES, NS, F], bf16)
    ag2_in_d = nc.dram_tensor("ag2_in", [R + 1, NS, F], bf16)
    gtab = nc.dram_tensor("gtab", [N_CORES * (R + 1) * NS, F], bf16)

    with tile.TileContext(nc) as tc:
        with (
            tc.tile_pool(name="adj", bufs=1) as adj_pool,
            tc.tile_pool(name="persist", bufs=1) as pp,
            tc.tile_pool(name="work", bufs=1) as wp,
            tc.tile_pool(name="psA", bufs=1, space="PSUM") as psA,
            tc.tile_pool(name="dram", bufs=1, space="DRAM") as dp,
        ):
            # ---------- constant / small loads ----------
            ones_bf = pp.tile([128, 128], bf16, tag="ones")
            nc.vector.memset(ones_bf[:], 1.0)

            # ---------- adjacency: 32 resident chunk tiles (queued first) ----------
            adj_tiles = []
            for k in range(KC):
                t = adj_pool.tile([128, R, NS], bf16, tag=f"adj{k}")
                nc.sync.dma_start(out=t[:], in_=adjt[k])
                adj_tiles.append(t)

            w1t_sb = pp.tile([128, R, F], f32, tag="w1t")
            nc.sync.dma_start(out=w1t_sb[:], in_=w1t[:].rearrange("r f o -> f r o"))
            w2t_sb = pp.tile([128, R, F], f32, tag="w2t")
            nc.sync.dma_start(out=w2t_sb[:], in_=w2t[:].rearrange("r f o -> f r o"))
            relm_sb = pp.tile([128, R + 1, F], f32, tag="relm")
            nc.sync.dma_start(out=relm_sb[:], in_=relm5[:].rearrange("t f g -> f t g"))

            e1_sb = pp.tile([128, BS // 128], i32, tag="e1")
            nc.sync.dma_start(out=e1_sb[:], in_=e1x[:].rearrange("(p j) -> p j", p=128))
            rel_sb = pp.tile([128, BS // 128], i32, tag="rel")
            nc.sync.dma_start(
                out=rel_sb[:], in_=relx[:].rearrange("(p j) -> p j", p=128)
            )
            e2_sb = pp.tile([128, BS // 128], i32, tag="e2")
            nc.sync.dma_start(out=e2_sb[:], in_=e2x[:].rearrange("(p j) -> p j", p=128))

            # features: f32 load (staged), cast to bf16 chunk tiles
            x_bf = pp.tile([128, KC, F], bf16, tag="xbf")
            feat_v = feat[:].rearrange("(k p) f -> p k f", p=128)
            for q in range(4):
                x_f32 = wp.tile([128, KC // 4, F], f32, tag="xf32", bufs=2)
                nc.sync.dma_start(
                    out=x_f32[:], in_=feat_v[:, q * (KC // 4) : (q + 1) * (KC // 4), :]
                )
                nc.vector.tensor_copy(
                    out=x_bf[:, q * (KC // 4) : (q + 1) * (KC // 4), :], in_=x_f32[:]
                )

            # ---------- layer 1 matmuls: aggT[r] and deg[r] ----------
            agg_ps = [
                psA.tile([128, NS], f32, tag=f"agg{r}", name=f"agg{r}")
                for r in range(R)
            ]
            psD_cm = tc.tile_pool(name="psD", bufs=1, space="PSUM")
            psD = psD_cm.__enter__()
            deg_ps = [
                psD.tile([128, NS], f32, tag=f"deg{r}", name=f"deg{r}")
                for r in range(R)
            ]
            for k in range(KC):
                for r in range(R):
                    nc.tensor.matmul(
                        deg_ps[r][:],
                        ones_bf[:],
                        adj_tiles[k][:, r, :],
                        start=(k == 0),
                        stop=(k == KC - 1),
                    )
                for r in range(R):
                    nc.tensor.matmul(
                        agg_ps[r][:],
                        x_bf[:, k, :],
                        adj_tiles[k][:, r, :],
                        start=(k == 0),
                        stop=(k == KC - 1),
                    )

            # ---------- 1/deg with one Newton step (deg==0 -> 1) ----------
            invd = pp.tile([128, R, NS], f32, tag="invd")
            ones_f = pp.tile([128, NS], f32, tag="onesf")
            nc.vector.memset(ones_f[:], 1.0)
            for r in range(R):
                nc.vector.tensor_tensor(
                    out=invd[:, r, :], in0=deg_ps[r][:], in1=ones_f[:], op=Alu.max
                )
                nc.vector.reciprocal(out=invd[:, r, :], in_=invd[:, r, :])
            psD_cm.__exit__(None, None, None)
            psB_cm = tc.tile_pool(name="psB", bufs=1, space="PSUM")
            psB = psB_cm.__enter__()

            def conv_tail(agg_banks, wt_sb, out_ht):
                """scale by invd, m2 accumulate over r, sigmoid -> out_ht f32."""
                y_ps = psB.tile([128, NS], f32, tag="y")
                for r in range(R):
                    aggs = wp.tile([128, NS], f32, tag="aggs", bufs=2, name="aggs")
                    nc.vector.tensor_tensor(
                        out=aggs[:],
                        in0=agg_banks[r][:],
                        in1=invd[:, r, :],
                        op=Alu.mult,
                    )
                    nc.tensor.matmul(
                        y_ps[:],
                        wt_sb[:, r, :],
                        aggs[:],
                        start=(r == 0),
                        stop=(r == R - 1),
                    )
                nc.scalar.activation(out=out_ht[:], in_=y_ps[:], func=Act.Sigmoid)

            # ---------- layer 1 tail + transpose + AllGather ----------
            h1t = wp.tile([128, NS], f32, tag="h1t")
            conv_tail(agg_ps, w1t_sb, h1t)

            s_sb = wp.tile([128, BS // 128], f32, tag="ssb")
            if stage >= 2:
                ident = relm_sb[:, R, :]  # [128,128] identity (host-provided)
                h1n = wp.tile([128, NB, F], bf16, tag="h1n")
                for nb in range(NB):
                    tr_ps = psB.tile([128, F], f32, tag="tr")
                    nc.tensor.transpose(
                        tr_ps[:], h1t[:, nb * 128 : (nb + 1) * 128], ident
                    )
                    nc.vector.tensor_copy(out=h1n[:, nb, :], in_=tr_ps[:])

                nc.sync.dma_start(
                    out=ag1_in_d[:].rearrange("(nb p) f -> p nb f", p=128),
                    in_=h1n[:],
                )
                nc.gpsimd.collective_compute(
                    "AllGather",
                    Alu.bypass,
                    replica_groups=[list(range(N_CORES))],
                    ins=[ag1_in_d[:]],
                    outs=[ag1_out_d[:]],
                )
                import os as _os

                n_fill = int(_os.environ.get("NFILL", "120"))
                if n_fill:
                    warm_ps = psB.tile([128, NS], f32, tag="warm")
                    for _i in range(n_fill):
                        nc.tensor.matmul(
                            warm_ps[:],
                            ones_bf[:],
                            adj_tiles[0][:, 0, :],
                            start=True,
                            stop=True,
                        )
                h1_all = pp.tile([128, KC, F], bf16, tag="h1all")
                nc.sync.dma_start(
                    out=h1_all[:],
                    in_=ag1_out_d[:].rearrange("c (q p) f -> p (c q) f", p=128),
                )

                # ---------- layer 2 ----------
                agg2_ps = [
                    psA.tile([128, NS], f32, tag=f"agg{r}", name=f"agg2_{r}")
                    for r in range(R)
                ]
                for k in range(KC):
                    for r in range(R):
                        nc.tensor.matmul(
                            agg2_ps[r][:],
                            h1_all[:, k, :],
                            adj_tiles[k][:, r, :],
                            start=(k == 0),
                            stop=(k == KC - 1),
                        )
                h2t = wp.tile([128, NS], f32, tag="h2t")
                conv_tail(agg2_ps, w2t_sb, h2t)

            if stage >= 3:
                # ---------- T tables: T[t] = H2_shard @ relm5[t] ----------
                for t in range(R + 1):
                    tt_sb = wp.tile([128, NB, F], bf16, tag="ttab", bufs=2)
                    for nb in range(NB):
                        t_ps = psB.tile([128, F], f32, tag="tps")
                        nc.tensor.matmul(
                            t_ps[:],
                            h2t[:, nb * 128 : (nb + 1) * 128],
                            relm_sb[:, t, :],
                            start=True,
                            stop=True,
                        )
                        nc.vector.tensor_copy(out=tt_sb[:, nb, :], in_=t_ps[:])
                    nc.sync.dma_start(
                        out=ag2_in_d[t].rearrange("(nb p) f -> p nb f", p=128),
                        in_=tt_sb[:],
                    )

                nc.gpsimd.collective_compute(
                    "AllGather",
                    Alu.bypass,
                    replica_groups=[list(range(N_CORES))],
                    ins=[ag2_in_d[:]],
                    outs=[gtab[:].rearrange("(c t n) f -> c t n f", c=N_CORES, t=R + 1)],
                )

            if stage >= 4:
                # ---------- DistMult batch shard ----------
                # row index in [(c t n), f] view: c*2560 + t*512 + n
                u_off = wp.tile([128, BS // 128], i32, tag="uoff")
                v_off = wp.tile([128, BS // 128], i32, tag="voff")
                t1 = wp.tile([128, BS // 128], i32, tag="it1")
                # u_off = (e1>>9)*2560 + (e1&511) + rel*512
                nc.vector.tensor_scalar(
                    out=u_off[:], in0=e1_sb[:], scalar1=9, scalar2=None,
                    op0=Alu.logical_shift_right,
                )
                nc.vector.tensor_scalar_mul(u_off[:], u_off[:], 2560)
                nc.vector.tensor_scalar(
                    out=t1[:], in0=e1_sb[:], scalar1=511, scalar2=None,
                    op0=Alu.bitwise_and,
                )
                nc.vector.tensor_tensor(
                    out=u_off[:], in0=u_off[:], in1=t1[:], op=Alu.add
                )
                nc.vector.tensor_scalar_mul(t1[:], rel_sb[:], 512)
                nc.vector.tensor_tensor(
                    out=u_off[:], in0=u_off[:], in1=t1[:], op=Alu.add
                )
                # v_off = (e2>>9)*2560 + (e2&511) + 4*512
                nc.vector.tensor_scalar(
                    out=v_off[:], in0=e2_sb[:], scalar1=9, scalar2=None,
                    op0=Alu.logical_shift_right,
                )
                nc.vector.tensor_scalar_mul(v_off[:], v_off[:], 2560)
                nc.vector.tensor_scalar(
                    out=t1[:], in0=e2_sb[:], scalar1=511, scalar2=None,
                    op0=Alu.bitwise_and,
                )
                nc.vector.tensor_tensor(
                    out=v_off[:], in0=v_off[:], in1=t1[:], op=Alu.add
                )
                nc.vector.tensor_scalar_add(v_off[:], v_off[:], 2048)

                table_rows = gtab[:]
                prod = wp.tile([128, F], f32, tag="prod")
                from concourse import bass as _bass

                for j in range(BS // 128):
                    u_t = wp.tile([128, F], bf16, tag="ut", bufs=2)
                    v_t = wp.tile([128, F], bf16, tag="vt", bufs=2)
                    nc.gpsimd.indirect_dma_start(
                        out=u_t[:],
                        out_offset=None,
                        in_=table_rows,
                        in_offset=_bass.IndirectOffsetOnAxis(
                            ap=u_off[:, j : j + 1], axis=0
                        ),
                    )
                    nc.gpsimd.indirect_dma_start(
                        out=v_t[:],
                        out_offset=None,
                        in_=table_rows,
                        in_offset=_bass.IndirectOffsetOnAxis(
                            ap=v_off[:, j : j + 1], axis=0
                        ),
                    )
                    nc.vector.tensor_tensor(
                        out=prod[:], in0=u_t[:], in1=v_t[:], op=Alu.mult
                    )
                    nc.vector.tensor_reduce(
                        out=s_sb[:, j : j + 1], in_=prod[:],
                        axis=mybir.AxisListType.X, op=Alu.add,
                    )
            elif stage == 3:
                g_t = wp.tile([128, BS // 128], bf16, tag="g3")
                nc.sync.dma_start(out=g_t[:], in_=gtab[:128, : BS // 128])
                nc.vector.tensor_copy(out=s_sb[:], in_=g_t[:])
            elif stage == 2:
                nc.vector.tensor_copy(out=s_sb[:], in_=h2t[:, : BS // 128])
            else:
                nc.vector.tensor_copy(out=s_sb[:], in_=h1t[:, : BS // 128])

            nc.sync.dma_start(
                out=scores[:].rearrange("(p j) -> p j", p=128), in_=s_sb[:]
            )
            psB_cm.__exit__(None, None, None)

    nc.compile()
    return nc


_STAGE = 4


def _get_nc():
    global _NC
    if _NC is None:
        _install_ntff_shim()
        _NC = _build(_STAGE)
    return _NC


def _prep_in_maps(features, adj, W1, W2, relmats, e1_idx, rel_idx, e2_idx):
    features = np.asarray(features, dtype=np.float32)
    adj = np.asarray(adj, dtype=np.float32)
    W1 = np.asarray(W1, dtype=np.float32)
    W2 = np.asarray(W2, dtype=np.float32)
    relmats = np.asarray(relmats, dtype=np.float32)
    e1 = np.asarray(e1_idx).astype(np.int32)
    rel = np.asarray(rel_idx).astype(np.int32)
    e2 = np.asarray(e2_idx).astype(np.int32)

    # adjT[m, r, n] = adj[r, n, m]; 0/1 values -> exact in bf16
    adjT = np.ascontiguousarray(adj.transpose(2, 0, 1)).astype(BF16)
    w1t = np.ascontiguousarray(W1.transpose(0, 2, 1))
    w2t = np.ascontiguousarray(W2.transpose(0, 2, 1))
    relm5 = np.concatenate(
        [relmats, np.eye(F, dtype=np.float32)[None]], axis=0
    ).astype(np.float32)

    in_maps = []
    for c in range(N_CORES):
        sl = slice(c * NS, (c + 1) * NS)
        bsl = slice(c * BS, (c + 1) * BS)
        adj_c = np.ascontiguousarray(adjT[:, :, sl]).reshape(KC, 128, R, NS)
        in_maps.append(
            {
                "adjt": adj_c,
                "feat": features,
                "w1t": w1t,
                "w2t": w2t,
                "relm5": relm5,
                "e1x": np.ascontiguousarray(e1[bsl]),
                "relx": np.ascontiguousarray(rel[bsl]),
                "e2x": np.ascontiguousarray(e2[bsl]),
            }
        )
    return in_maps


def kernel(
    features, adj, W1, W2, relmats, e1_idx, rel_idx, e2_idx, _trace=False
):
    from concourse.bass_utils import run_bass_kernel_spmd

    nc = _get_nc()
    in_maps = _prep_in_maps(
        features, adj, W1, W2, relmats, e1_idx, rel_idx, e2_idx
    )
    try:
        res = run_bass_kernel_spmd(
            nc, in_maps, list(range(N_CORES)), trace=_trace
        )
    except Exception:
        # transient NRT device errors recover on retry
        res = run_bass_kernel_spmd(
            nc, in_maps, list(range(N_CORES)), trace=_trace
        )
    out = np.concatenate([res.results[c]["scores"] for c in range(N_CORES)])
    if _trace:
        kernel.last_results = res
    return out


# revision 22
# speedup vs baseline: 1.2946x; 1.1738x over previous
"""BasicRGCN (2-layer R-GCN conv + DistMult scoring) on 8 Trainium2 NeuronCores.

Sharding: entity rows N=4096 are split 512/core for the conv layers (the
[R,N,N] adjacency is the only large tensor; each core streams its row shard
as the matmul moving operand in transposed layout). Between layers the h
shards are AllGathered. The DistMult stage builds per-core shards of the
tables T[r] = H2 @ M_r (plus T[4] = H2), AllGathers them, and each core
resolves its 1024-sample batch shard with indirect-DMA row gathers.

Dataflow per core (SPMD, same program, different data):
  adjT chunks [32][128m, 4r, 512n] bf16 resident in SBUF (16 MiB)
  L1: aggT[r] = X_k.T @ adjT (PSUM accum over 32 k-chunks), deg via ones lhsT
      y.T = sum_r W1[r].T.T @ (aggT[r] * 1/deg[r]);  h1.T = sigmoid(y.T)
      PE-transpose -> h1 [512,128] bf16 -> AllGather -> H1 [4096,128]
  L2: same with H1/W2 -> h2.T [128f, 512n] f32
  T:  T[t][n,g] = h2T[:,nb].T @ relm5[t]  (relm5[4] = I) -> AllGather
  DistMult: row gathers from T_full by (rel,e1) and e2, fused mul+reduce.
"""

import sys
import types

import ml_dtypes
import numpy as np

N_CORES = 8
N_ENT = 4096
R = 4
F = 128
BATCH = 8192
NS = N_ENT // N_CORES  # 512 entity rows per core
BS = BATCH // N_CORES  # 1024 batch samples per core
BS_PAD = 1408  # per-core batch capacity after owner(e1) routing (11 * 128)
BSJ = BS_PAD // 128
KC = N_ENT // 128  # 32 contraction chunks
NB = NS // 128  # 4 row blocks per core
BF16 = ml_dtypes.bfloat16


def _install_ntff_shim():
    """Agent image's antenv lacks axon_hooks; recreate it from the boot's
    ctypes NTFF driver so trace=True profiling works when requested."""
    if "antenv.axon_hooks" in sys.modules:
        return
    try:
        import antenv
        from trn_agent_boot.trn_boot import _ntff_profile_via_ctypes

        hook = _ntff_profile_via_ctypes("/opt/axon/libaxon_pjrt.so")
        mod = types.ModuleType("antenv.axon_hooks")
        state = {"hook": hook}
        mod.get_axon_ntff_profile_hook = lambda: state["hook"]
        mod.set_axon_ntff_profile_hook = lambda h: state.__setitem__("hook", h)
        sys.modules["antenv.axon_hooks"] = mod
        antenv.axon_hooks = mod
    except Exception:
        pass


_NC = None


def _build(stage=4):
    from concourse import bacc, tile
    import concourse.mybir as mybir

    f32 = mybir.dt.float32
    bf16 = mybir.dt.bfloat16
    i32 = mybir.dt.int32
    Alu = mybir.AluOpType
    Act = mybir.ActivationFunctionType

    nc = bacc.Bacc(
        "TRN2", target_bir_lowering=False, debug=False, num_devices=N_CORES
    )

    adjt = nc.dram_tensor("adjt", [KC, 128, R, NS], bf16, kind="ExternalInput")
    feat = nc.dram_tensor("feat", [N_ENT, F], f32, kind="ExternalInput")
    w1t = nc.dram_tensor("w1t", [R, F, F], f32, kind="ExternalInput")
    w2t = nc.dram_tensor("w2t", [R, F, F], f32, kind="ExternalInput")
    relm5 = nc.dram_tensor("relm5", [R + 1, F, F], f32, kind="ExternalInput")
    e1x = nc.dram_tensor("e1x", [BS_PAD], i32, kind="ExternalInput")
    relx = nc.dram_tensor("relx", [BS_PAD], i32, kind="ExternalInput")
    e2x = nc.dram_tensor("e2x", [BS_PAD], i32, kind="ExternalInput")
    scores = nc.dram_tensor("scores", [BS_PAD], f32, kind="ExternalOutput")

    ag1_in_d = nc.dram_tensor("ag1_in", [NS, F], bf16)
    ag1_out_d = nc.dram_tensor("ag1_out", [N_CORES, NS, F], bf16)
    ag2_in_d = nc.dram_tensor("ag2_in", [NS, F], bf16)
    h2full_d = nc.dram_tensor("h2full", [N_CORES * NS, F], bf16)
    zloc_d = nc.dram_tensor("zloc", [R * NS, F], bf16)

    with tile.TileContext(nc) as tc:
        with (
            tc.tile_pool(name="adj", bufs=1) as adj_pool,
            tc.tile_pool(name="persist", bufs=1) as pp,
            tc.tile_pool(name="work", bufs=1) as wp,
            tc.tile_pool(name="psA", bufs=1, space="PSUM") as psA,
            tc.tile_pool(name="dram", bufs=1, space="DRAM") as dp,
        ):
            # ---------- constant / small loads ----------
            ones_bf = pp.tile([128, 128], bf16, tag="ones")
            nc.vector.memset(ones_bf[:], 1.0)

            # ---------- adjacency: 32 resident chunk tiles (queued first) ----------
            adj_tiles = []
            for k in range(KC):
                t = adj_pool.tile([128, R, NS], bf16, tag=f"adj{k}")
                nc.sync.dma_start(out=t[:], in_=adjt[k])
                adj_tiles.append(t)

            w1t_sb = pp.tile([128, R, F], f32, tag="w1t")
            nc.sync.dma_start(out=w1t_sb[:], in_=w1t[:].rearrange("r f o -> f r o"))
            w2t_sb = pp.tile([128, R, F], f32, tag="w2t")
            nc.sync.dma_start(out=w2t_sb[:], in_=w2t[:].rearrange("r f o -> f r o"))
            relm_sb = pp.tile([128, R + 1, F], f32, tag="relm")
            nc.sync.dma_start(out=relm_sb[:], in_=relm5[:].rearrange("t f g -> f t g"))

            e1_sb = pp.tile([128, BSJ], i32, tag="e1")
            nc.sync.dma_start(out=e1_sb[:], in_=e1x[:].rearrange("(p j) -> p j", p=128))
            rel_sb = pp.tile([128, BSJ], i32, tag="rel")
            nc.sync.dma_start(
                out=rel_sb[:], in_=relx[:].rearrange("(p j) -> p j", p=128)
            )
            e2_sb = pp.tile([128, BSJ], i32, tag="e2")
            nc.sync.dma_start(out=e2_sb[:], in_=e2x[:].rearrange("(p j) -> p j", p=128))

            # features: f32 load (staged), cast to bf16 chunk tiles
            x_bf = pp.tile([128, KC, F], bf16, tag="xbf")
            feat_v = feat[:].rearrange("(k p) f -> p k f", p=128)
            for q in range(4):
                x_f32 = wp.tile([128, KC // 4, F], f32, tag="xf32", bufs=2)
                nc.sync.dma_start(
                    out=x_f32[:], in_=feat_v[:, q * (KC // 4) : (q + 1) * (KC // 4), :]
                )
                nc.vector.tensor_copy(
                    out=x_bf[:, q * (KC // 4) : (q + 1) * (KC // 4), :], in_=x_f32[:]
                )

            # ---------- layer 1 matmuls: aggT[r] and deg[r] ----------
            agg_ps = [
                psA.tile([128, NS], f32, tag=f"agg{r}", name=f"agg{r}")
                for r in range(R)
            ]
            psD_cm = tc.tile_pool(name="psD", bufs=1, space="PSUM")
            psD = psD_cm.__enter__()
            deg_ps = [
                psD.tile([128, NS], f32, tag=f"deg{r}", name=f"deg{r}")
                for r in range(R)
            ]
            for k in range(KC):
                for r in range(R):
                    nc.tensor.matmul(
                        deg_ps[r][:],
                        ones_bf[:],
                        adj_tiles[k][:, r, :],
                        start=(k == 0),
                        stop=(k == KC - 1),
                    )
                for r in range(R):
                    nc.tensor.matmul(
                        agg_ps[r][:],
                        x_bf[:, k, :],
                        adj_tiles[k][:, r, :],
                        start=(k == 0),
                        stop=(k == KC - 1),
                    )

            # ---------- 1/deg with one Newton step (deg==0 -> 1) ----------
            invd = pp.tile([128, R, NS], f32, tag="invd")
            ones_f = pp.tile([128, NS], f32, tag="onesf")
            nc.vector.memset(ones_f[:], 1.0)
            for r in range(R):
                nc.vector.tensor_tensor(
                    out=invd[:, r, :], in0=deg_ps[r][:], in1=ones_f[:], op=Alu.max
                )
                nc.vector.reciprocal(out=invd[:, r, :], in_=invd[:, r, :])
            psD_cm.__exit__(None, None, None)
            psB_cm = tc.tile_pool(name="psB", bufs=1, space="PSUM")
            psB = psB_cm.__enter__()

            def conv_tail(agg_banks, wt_sb, out_ht):
                """scale by invd, m2 accumulate over r, sigmoid -> out_ht f32."""
                y_ps = psB.tile([128, NS], f32, tag="y")
                for r in range(R):
                    aggs = wp.tile([128, NS], f32, tag="aggs", bufs=2, name="aggs")
                    nc.vector.tensor_tensor(
                        out=aggs[:],
                        in0=agg_banks[r][:],
                        in1=invd[:, r, :],
                        op=Alu.mult,
                    )
                    nc.tensor.matmul(
                        y_ps[:],
                        wt_sb[:, r, :],
                        aggs[:],
                        start=(r == 0),
                        stop=(r == R - 1),
                    )
                nc.scalar.activation(out=out_ht[:], in_=y_ps[:], func=Act.Sigmoid)

            # ---------- layer 1 tail + transpose + AllGather ----------
            h1t = wp.tile([128, NS], f32, tag="h1t")
            conv_tail(agg_ps, w1t_sb, h1t)

            s_sb = wp.tile([128, BSJ], f32, tag="ssb")
            if stage >= 2:
                ident = relm_sb[:, R, :]  # [128,128] identity (host-provided)
                h1n = wp.tile([128, NB, F], bf16, tag="h1n")
                for nb in range(NB):
                    tr_ps = psB.tile([128, F], f32, tag="tr")
                    nc.tensor.transpose(
                        tr_ps[:], h1t[:, nb * 128 : (nb + 1) * 128], ident
                    )
                    nc.vector.tensor_copy(out=h1n[:, nb, :], in_=tr_ps[:])

                nc.sync.dma_start(
                    out=ag1_in_d[:].rearrange("(nb p) f -> p nb f", p=128),
                    in_=h1n[:],
                )
                nc.gpsimd.collective_compute(
                    "AllGather",
                    Alu.bypass,
                    replica_groups=[list(range(N_CORES))],
                    ins=[ag1_in_d[:]],
                    outs=[ag1_out_d[:]],
                )
                import os as _os

                n_fill = int(_os.environ.get("NFILL", "120"))
                if n_fill:
                    warm_ps = psB.tile([128, NS], f32, tag="warm")
                    for _i in range(n_fill):
                        nc.tensor.matmul(
                            warm_ps[:],
                            ones_bf[:],
                            adj_tiles[0][:, 0, :],
                            start=True,
                            stop=True,
                        )
                h1_all = pp.tile([128, KC, F], bf16, tag="h1all")
                nc.sync.dma_start(
                    out=h1_all[:],
                    in_=ag1_out_d[:].rearrange("c (q p) f -> p (c q) f", p=128),
                )

                # ---------- layer 2 ----------
                agg2_ps = [
                    psA.tile([128, NS], f32, tag=f"agg{r}", name=f"agg2_{r}")
                    for r in range(R)
                ]
                for k in range(KC):
                    for r in range(R):
                        nc.tensor.matmul(
                            agg2_ps[r][:],
                            h1_all[:, k, :],
                            adj_tiles[k][:, r, :],
                            start=(k == 0),
                            stop=(k == KC - 1),
                        )
                h2t = wp.tile([128, NS], f32, tag="h2t")
                conv_tail(agg2_ps, w2t_sb, h2t)

            if stage >= 3:
                # ---------- h2 natural-layout AllGather + local Z tables ----------
                h2n = wp.tile([128, NB, F], bf16, tag="h1n")
                for nb in range(NB):
                    tr_ps = psB.tile([128, F], f32, tag="tr")
                    nc.tensor.transpose(
                        tr_ps[:], h2t[:, nb * 128 : (nb + 1) * 128], ident
                    )
                    nc.vector.tensor_copy(out=h2n[:, nb, :], in_=tr_ps[:])
                nc.sync.dma_start(
                    out=ag2_in_d[:].rearrange("(nb p) f -> p nb f", p=128),
                    in_=h2n[:],
                )
                nc.gpsimd.collective_compute(
                    "AllGather",
                    Alu.bypass,
                    replica_groups=[list(range(N_CORES))],
                    ins=[ag2_in_d[:]],
                    outs=[h2full_d[:].rearrange("(c n) f -> c n f", c=N_CORES)],
                )
                # Z[r] = h2_shard @ relm[r] -> local dram [R*NS, F] bf16
                for t in range(R):
                    zt_sb = wp.tile([128, NB, F], bf16, tag="ttab", bufs=2)
                    for nb in range(NB):
                        t_ps = psB.tile([128, F], f32, tag="tps")
                        nc.tensor.matmul(
                            t_ps[:],
                            h2t[:, nb * 128 : (nb + 1) * 128],
                            relm_sb[:, t, :],
                            start=True,
                            stop=True,
                        )
                        nc.vector.tensor_copy(out=zt_sb[:, nb, :], in_=t_ps[:])
                    nc.sync.dma_start(
                        out=zloc_d[t * NS : (t + 1) * NS].rearrange(
                            "(nb p) f -> p nb f", p=128
                        ),
                        in_=zt_sb[:],
                    )

            if stage >= 4:
                # ---------- DistMult: u from local Z, v from gathered H2 ----------
                u_off = wp.tile([128, BSJ], i32, tag="uoff")
                t1 = wp.tile([128, BSJ], i32, tag="it1")
                # u_off = rel*512 + e1loc   (e1loc = e1 & 511, host-permuted)
                nc.vector.tensor_scalar_mul(u_off[:], rel_sb[:], NS)
                nc.vector.tensor_tensor(
                    out=u_off[:], in0=u_off[:], in1=e1_sb[:], op=Alu.add
                )
                prod = wp.tile([128, F], f32, tag="prod")
                from concourse import bass as _bass

                for j in range(BSJ):
                    u_t = wp.tile([128, F], bf16, tag="ut", bufs=2)
                    v_t = wp.tile([128, F], bf16, tag="vt", bufs=2)
                    nc.gpsimd.indirect_dma_start(
                        out=u_t[:],
                        out_offset=None,
                        in_=zloc_d[:],
                        in_offset=_bass.IndirectOffsetOnAxis(
                            ap=u_off[:, j : j + 1], axis=0
                        ),
                    )
                    nc.gpsimd.indirect_dma_start(
                        out=v_t[:],
                        out_offset=None,
                        in_=h2full_d[:],
                        in_offset=_bass.IndirectOffsetOnAxis(
                            ap=e2_sb[:, j : j + 1], axis=0
                        ),
                    )
                    nc.vector.tensor_tensor(
                        out=prod[:], in0=u_t[:], in1=v_t[:], op=Alu.mult
                    )
                    nc.vector.tensor_reduce(
                        out=s_sb[:, j : j + 1], in_=prod[:],
                        axis=mybir.AxisListType.X, op=Alu.add,
                    )
            elif stage == 3:
                g_t = wp.tile([128, BSJ], bf16, tag="g3")
                nc.sync.dma_start(out=g_t[:], in_=h2full_d[:128, :BSJ])
                nc.vector.tensor_copy(out=s_sb[:], in_=g_t[:])
            elif stage == 2:
                nc.vector.tensor_copy(out=s_sb[:], in_=h2t[:, :BSJ])
            else:
                nc.vector.tensor_copy(out=s_sb[:], in_=h1t[:, :BSJ])

            nc.sync.dma_start(
                out=scores[:].rearrange("(p j) -> p j", p=128), in_=s_sb[:]
            )
            psB_cm.__exit__(None, None, None)

    nc.compile()
    return nc


_STAGE = 4


def _get_nc():
    global _NC
    if _NC is None:
        _install_ntff_shim()
        _NC = _build(_STAGE)
    return _NC


def _prep_in_maps(features, adj, W1, W2, relmats, e1_idx, rel_idx, e2_idx):
    features = np.asarray(features, dtype=np.float32)
    adj = np.asarray(adj, dtype=np.float32)
    W1 = np.asarray(W1, dtype=np.float32)
    W2 = np.asarray(W2, dtype=np.float32)
    relmats = np.asarray(relmats, dtype=np.float32)
    e1 = np.asarray(e1_idx).astype(np.int32)
    rel = np.asarray(rel_idx).astype(np.int32)
    e2 = np.asarray(e2_idx).astype(np.int32)

    # adjT[m, r, n] = adj[r, n, m]; 0/1 values -> exact in bf16
    adjT = np.ascontiguousarray(adj.transpose(2, 0, 1)).astype(BF16)
    w1t = np.ascontiguousarray(W1.transpose(0, 2, 1))
    w2t = np.ascontiguousarray(W2.transpose(0, 2, 1))
    relm5 = np.concatenate(
        [relmats, np.eye(F, dtype=np.float32)[None]], axis=0
    ).astype(np.float32)

    # route each sample to the core that owns entity e1 (u-gathers stay local)
    owner = e1 >> 9
    perms = [np.nonzero(owner == c)[0] for c in range(N_CORES)]
    counts = [len(p) for p in perms]
    if max(counts) > BS_PAD:
        raise ValueError(f"batch shard overflow: {max(counts)} > {BS_PAD}")

    in_maps = []
    for c in range(N_CORES):
        sl = slice(c * NS, (c + 1) * NS)
        adj_c = np.ascontiguousarray(adjT[:, :, sl]).reshape(KC, 128, R, NS)
        p = perms[c]
        e1p = np.zeros(BS_PAD, np.int32)
        relp = np.zeros(BS_PAD, np.int32)
        e2p = np.zeros(BS_PAD, np.int32)
        e1p[: len(p)] = e1[p] & (NS - 1)
        relp[: len(p)] = rel[p]
        e2p[: len(p)] = e2[p]
        in_maps.append(
            {
                "adjt": adj_c,
                "feat": features,
                "w1t": w1t,
                "w2t": w2t,
                "relm5": relm5,
                "e1x": e1p,
                "relx": relp,
                "e2x": e2p,
            }
        )
    return in_maps, perms


def kernel(
    features, adj, W1, W2, relmats, e1_idx, rel_idx, e2_idx, _trace=False
):
    from concourse.bass_utils import run_bass_kernel_spmd

    nc = _get_nc()
    in_maps, perms = _prep_in_maps(
        features, adj, W1, W2, relmats, e1_idx, rel_idx, e2_idx
    )
    try:
        res = run_bass_kernel_spmd(
            nc, in_maps, list(range(N_CORES)), trace=_trace
        )
    except Exception:
        # transient NRT device errors recover on retry
        res = run_bass_kernel_spmd(
            nc, in_maps, list(range(N_CORES)), trace=_trace
        )
    out = np.empty(BATCH, np.float32)
    for c in range(N_CORES):
        p = perms[c]
        out[p] = res.results[c]["scores"][: len(p)]
    if _trace:
        kernel.last_results = res
    return out
